# revision 4
# baseline (speedup 1.0000x reference)
"""2-layer GAT masked-autoencoder MSE, fully on 8 Trainium2 cores.

Single Bass/Tile SPMD program:
  - nodes sharded by range (12500/core); edges assigned to the core
    owning their dst, sorted by dst, grouped into 128-dst-node blocks
  - per layer: dense matmul h@[W|wal|war] -> per-node (f, el, er);
    AllGather of the [N,136] feature table; per 128-edge tile an
    indirect-DMA gather of src rows + one-hot selection matmul does
    edge softmax numerators and denominators in one pass
  - decoder matmul + masked-MSE partial sums fused into layer-2 epilogue
Host only packs index tensors (cached by content fingerprint) and
averages the 8x128 partial sums.
"""

import sys
import hashlib
import math
import numpy as np

for _p in ("/opt/trn_rl_repo", "/root/.axon_site/_ro/trn_rl_repo"):
    if _p not in sys.path:
        sys.path.append(_p)

N = 100000
E = 1600000
IN_DIM = 128
HID = 128
HEADS = 4
D = HID // HEADS
NEG_SLOPE = 0.2
NCORES = 8
ROWS = N // NCORES          # 12500
P = 128
NBLK = (ROWS + P - 1) // P  # 98
FE = HID + 2 * HEADS        # 136: [f(128) | el(4) | er(4)]
MASK_N = 30000

_CACHE = {}


def _fingerprint(*arrs):
    import zlib
    parts = []
    for a in arrs:
        a = np.ascontiguousarray(a)
        parts.append((a.shape, str(a.dtype),
                      zlib.crc32(memoryview(a.reshape(-1).view(np.uint8)))))
    return tuple(parts)


# ----------------------------------------------------------------- host prep

def _prep_graph(src, dst, mask_idx):
    """Pack edges into per-core [128, TT] tiles grouped by dst block."""
    core_of = dst // ROWS
    loc = dst - core_of * ROWS
    blk = loc >> 7
    rel = (loc & 127).astype(np.float32)

    order = np.argsort(dst, kind="stable")
    src_s = src[order].astype(np.int32)
    core_s = core_of[order]
    blk_s = blk[order]
    rel_s = rel[order]

    flat = (core_s * NBLK + blk_s).astype(np.int64)
    gcounts = np.bincount(flat, minlength=NCORES * NBLK)
    # tiles per block: max over cores, >=1
    cnt_cb = gcounts.reshape(NCORES, NBLK)
    T = np.maximum(1, (cnt_cb.max(axis=0) + P - 1) // P).astype(np.int64)
    TT = int(T.sum())
    tile_base = np.zeros(NBLK, np.int64)
    tile_base[1:] = np.cumsum(T)[:-1]

    gstart = np.zeros(NCORES * NBLK + 1, np.int64)
    gstart[1:] = np.cumsum(gcounts)
    k = np.arange(src_s.shape[0], dtype=np.int64) - gstart[flat]
    pos = tile_base[blk_s] * P + k            # slot within core's packed array

    srcpk = np.zeros((NCORES, TT * P), np.int32)
    dstpk = np.full((NCORES, TT * P), 999.0, np.float32)
    srcpk[core_s, pos] = src_s
    dstpk[core_s, pos] = rel_s
    # [TT*P] -> [128, TT] (partition p = edge p of tile t)
    srcpk = np.ascontiguousarray(
        srcpk.reshape(NCORES, TT, P).transpose(0, 2, 1))
    dstpk = np.ascontiguousarray(
        dstpk.reshape(NCORES, TT, P).transpose(0, 2, 1))

    # masks
    m_core = (mask_idx // ROWS).astype(np.int64)
    m_loc = (mask_idx - m_core * ROWS).astype(np.int32)
    mc = np.bincount(m_core, minlength=NCORES)
    TM = int((mc.max() + P - 1) // P)
    maskpk = np.zeros((NCORES, TM * P), np.int32)
    maskval = np.zeros((NCORES, TM * P), np.float32)
    mask_rows_sel = np.zeros((NCORES, TM * P), np.int64)  # global idx for attr row fetch
    morder = np.argsort(m_core, kind="stable")
    ms_core = m_core[morder]
    ms_loc = m_loc[morder]
    ms_glob = mask_idx[morder].astype(np.int64)
    mstart = np.zeros(NCORES + 1, np.int64)
    mstart[1:] = np.cumsum(mc)
    mk = np.arange(mask_idx.shape[0], dtype=np.int64) - mstart[ms_core]
    maskpk[ms_core, mk] = ms_loc
    maskval[ms_core, mk] = 1.0
    mask_rows_sel[ms_core, mk] = ms_glob
    maskval_flat = maskval.copy()
    maskpk = np.ascontiguousarray(maskpk.reshape(NCORES, TM, P).transpose(0, 2, 1))
    maskval = np.ascontiguousarray(maskval.reshape(NCORES, TM, P).transpose(0, 2, 1))

    return dict(T=tuple(int(x) for x in T), TT=TT, TM=TM,
                srcpk=srcpk, dstpk=dstpk,
                maskpk=maskpk, maskval=maskval, mask_rows_sel=mask_rows_sel,
                maskval_flat=maskval_flat)


# ------------------------------------------------------------- program build

def _build_program(T, TT, TM, ncores=None, stop_after="full"):
    if ncores is None:
        ncores = NCORES
    from concourse import bacc, bass, mybir
    from concourse import tile
    from concourse.masks import make_identity

    f32 = mybir.dt.float32
    i32 = mybir.dt.int32
    IOA = bass.IndirectOffsetOnAxis
    OP = mybir.AluOpType
    ACT = mybir.ActivationFunctionType
    T_MAX = max(T)

    nc = bacc.Bacc()

    # ---- I/O
    xT = nc.declare_dram_parameter("xt", [P, ROWS], f32, isOutput=False)
    wext0 = nc.declare_dram_parameter("wext0", [P, FE], f32, isOutput=False)
    wext1 = nc.declare_dram_parameter("wext1", [P, FE], f32, isOutput=False)
    wd = nc.declare_dram_parameter("wd", [P, HID], f32, isOutput=False)
    brep0 = nc.declare_dram_parameter("brep0", [P, HID], f32, isOutput=False)
    brep1 = nc.declare_dram_parameter("brep1", [P, HID], f32, isOutput=False)
    iota_in = nc.declare_dram_parameter("iota", [P, P], f32, isOutput=False)
    srcpk = nc.declare_dram_parameter("srcpk", [P, TT], i32, isOutput=False)
    dstpk = nc.declare_dram_parameter("dstpk", [P, TT], f32, isOutput=False)
    maskpk = nc.declare_dram_parameter("maskpk", [P, TM], i32, isOutput=False)
    maskval = nc.declare_dram_parameter("maskval", [P, TM], f32, isOutput=False)
    amask = nc.declare_dram_parameter("amask", [TM * P, IN_DIM], f32, isOutput=False)
    msepart = nc.declare_dram_parameter("msepart", [P, 1], f32, isOutput=True)

    # ---- DRAM scratch
    fext_shard = nc.dram_tensor("fext_shard", [ROWS, FE], f32, kind="Internal")
    fext_full = nc.dram_tensor("fext_full", [N, FE], f32, kind="Internal",
                               addr_space="Shared")
    h1T = nc.dram_tensor("h1T", [P, ROWS], f32, kind="Internal")
    recon = nc.dram_tensor("recon", [ROWS, HID], f32, kind="Internal")

    RG = [list(range(ncores))]

    with tile.TileContext(nc) as tc:
        with (
            tc.tile_pool(name="const", bufs=1) as cpool,
            tc.tile_pool(name="wrow", bufs=1) as wpool,
            tc.tile_pool(name="xin", bufs=3) as xpool,
            tc.tile_pool(name="fout", bufs=3) as fpool,
            tc.tile_pool(name="eres", bufs=1) as erpool,
            tc.tile_pool(name="blkidx", bufs=2) as bipool,
            tc.tile_pool(name="fx", bufs=T_MAX + 3) as fxpool,
            tc.tile_pool(name="smat", bufs=T_MAX + 3) as spool,
            tc.tile_pool(name="stmat", bufs=3) as stpool,
            tc.tile_pool(name="estg", bufs=2) as epool,
            tc.tile_pool(name="rhs", bufs=3) as rhspool,
            tc.tile_pool(name="accb", bufs=2) as accpool,
            tc.tile_pool(name="epi", bufs=3) as epipool,
            tc.tile_pool(name="mse", bufs=3) as msepool,
            tc.tile_pool(name="ps_t", bufs=2, space="PSUM") as pst,
            tc.tile_pool(name="ps_er", bufs=2, space="PSUM") as pser,
            tc.tile_pool(name="ps_mm", bufs=2, space="PSUM") as psmm,
            tc.tile_pool(name="ps_epi", bufs=2, space="PSUM") as psepi,
        ):
            ident = cpool.tile([P, P], f32, tag="ident")
            make_identity(nc, ident[:])
            iota_sb = cpool.tile([P, P], f32, tag="iota")
            nc.sync.dma_start(out=iota_sb[:], in_=iota_in[:])
            wd_sb = cpool.tile([P, HID], f32, tag="wd")
            nc.sync.dma_start(out=wd_sb[:], in_=wd[:])
            brep_sb = [cpool.tile([P, HID], f32, tag=f"brep{l}", name=f"brep_sb{l}")
                       for l in range(2)]
            nc.sync.dma_start(out=brep_sb[0][:], in_=brep0[:])
            nc.sync.dma_start(out=brep_sb[1][:], in_=brep1[:])
            wext_sb = [cpool.tile([P, FE], f32, tag=f"wext{l}", name=f"wext_sb{l}")
                       for l in range(2)]
            nc.sync.dma_start(out=wext_sb[0][:], in_=wext0[:])
            nc.sync.dma_start(out=wext_sb[1][:], in_=wext1[:])

            er_res = erpool.tile([P, HEADS * NBLK], f32, tag="er_res")

            def dense_phase(layer, lhsT_dram):
                """f_ext = h @ Wext; writes fext_shard + er_res."""
                nc.vector.memset(er_res[:], 0.0)
                for b in range(NBLK):
                    r0 = b * P
                    rw = min(P, ROWS - r0)
                    xt_t = xpool.tile([P, P], f32, tag="xt")
                    nc.sync.dma_start(out=xt_t[:, :rw], in_=lhsT_dram[:, r0:r0 + rw])
                    ps = psmm.tile([P, FE], f32, tag="mm")
                    nc.tensor.matmul(ps[:rw, :], lhsT=xt_t[:, :rw], rhs=wext_sb[layer][:],
                                     start=True, stop=True)
                    f_t = fpool.tile([P, FE], f32, tag="f")
                    nc.vector.tensor_copy(f_t[:rw, :], ps[:rw, :])
                    nc.vector.tensor_copy(er_res[:rw, HEADS * b:HEADS * (b + 1)],
                                          f_t[:rw, HID + HEADS:HID + 2 * HEADS])
                    nc.sync.dma_start(out=fext_shard[r0:r0 + rw, :], in_=f_t[:rw, :])

            def edge_phase(layer):
                """Returns per-block SBUF h tiles via callback epilogue."""
                tglob = 0
                for b in range(NBLK):
                    r0 = b * P
                    rw = min(P, ROWS - r0)
                    Tb = T[b]
                    sidx = bipool.tile([P, T_MAX], i32, tag="sidx")
                    nc.sync.dma_start(out=sidx[:, :Tb], in_=srcpk[:, tglob:tglob + Tb])
                    drel = bipool.tile([P, T_MAX], f32, tag="drel")
                    nc.sync.dma_start(out=drel[:, :Tb], in_=dstpk[:, tglob:tglob + Tb])

                    estage = epool.tile([P, HEADS * T_MAX], f32, tag="estage")
                    fx_list, s_list = [], []
                    for t in range(Tb):
                        fx = fxpool.tile([P, FE], f32, tag="fx")
                        nc.gpsimd.indirect_dma_start(
                            out=fx[:], out_offset=None,
                            in_=fext_full[:, :],
                            in_offset=IOA(ap=sidx[:, t:t + 1], axis=0))
                        S = spool.tile([P, P], f32, tag="S")
                        nc.vector.tensor_tensor(
                            out=S[:], in0=drel[:, t:t + 1].to_broadcast([P, P]),
                            in1=iota_sb[:], op=OP.is_equal)
                        ps_t_ = pst.tile([P, P], f32, tag="pst")
                        nc.tensor.transpose(ps_t_[:], S[:], ident[:])
                        sT = stpool.tile([P, P], f32, tag="sT")
                        nc.vector.tensor_copy(sT[:], ps_t_[:])
                        ps_e = pser.tile([P, HEADS], f32, tag="pser")
                        nc.tensor.matmul(ps_e[:], lhsT=sT[:],
                                         rhs=er_res[:, HEADS * b:HEADS * (b + 1)],
                                         start=True, stop=True)
                        nc.vector.tensor_tensor(
                            out=estage[:, HEADS * t:HEADS * (t + 1)],
                            in0=fx[:, HID:HID + HEADS], in1=ps_e[:], op=OP.add)
                        fx_list.append(fx)
                        s_list.append(S)
                    # leaky-relu + exp for the whole block in 2 ops
                    nc.vector.scalar_tensor_tensor(
                        out=estage[:, :HEADS * Tb], in0=estage[:, :HEADS * Tb],
                        scalar=NEG_SLOPE, in1=estage[:, :HEADS * Tb],
                        op0=OP.mult, op1=OP.max)
                    exstage = epool.tile([P, HEADS * T_MAX], f32, tag="exstage")
                    nc.scalar.activation(exstage[:, :HEADS * Tb],
                                         estage[:, :HEADS * Tb], ACT.Exp)

                    acc = accpool.tile([P, FE - HEADS], f32, tag="acc")
                    for t in range(Tb):
                        rhs = rhspool.tile([P, FE - HEADS], f32, tag="rhs")
                        for h in range(HEADS):
                            nc.vector.tensor_scalar(
                                rhs[:, D * h:D * (h + 1)],
                                fx_list[t][:, D * h:D * (h + 1)],
                                exstage[:, HEADS * t + h:HEADS * t + h + 1],
                                None, OP.mult)
                        nc.vector.tensor_copy(rhs[:, HID:HID + HEADS],
                                              exstage[:, HEADS * t:HEADS * (t + 1)])
                        ps = psmm.tile([P, FE], f32, tag="mm")
                        nc.tensor.matmul(ps[:, :FE - HEADS], lhsT=s_list[t][:],
                                         rhs=rhs[:], start=True, stop=True)
                        if t == 0:
                            nc.vector.tensor_copy(acc[:], ps[:, :FE - HEADS])
                        else:
                            nc.vector.tensor_add(acc[:], acc[:], ps[:, :FE - HEADS])
                    tglob += Tb

                    # epilogue: h = relu(num/den + b)
                    rec = epipool.tile([P, HEADS], f32, tag="rec")
                    nc.vector.tensor_scalar(rec[:], acc[:, HID:HID + HEADS],
                                            1e-20, None, OP.add)
                    nc.vector.reciprocal(rec[:], rec[:])
                    hrow = epipool.tile([P, HID], f32, tag="hrow")
                    for h in range(HEADS):
                        nc.vector.scalar_tensor_tensor(
                            out=hrow[:, D * h:D * (h + 1)],
                            in0=acc[:, D * h:D * (h + 1)],
                            scalar=rec[:, h:h + 1],
                            in1=brep_sb[layer][:, D * h:D * (h + 1)],
                            op0=OP.mult, op1=OP.add)
                    hrow2 = epipool.tile([P, HID], f32, tag="hrow2")
                    nc.scalar.activation(hrow2[:], hrow[:], ACT.Relu)
                    # transpose h rows -> [feat, node]
                    ps_tr = pst.tile([P, P], f32, tag="pst")
                    nc.tensor.transpose(ps_tr[:, :rw], hrow2[:rw, :],
                                        ident[:rw, :rw])
                    hT_sb = epipool.tile([P, P], f32, tag="hT")
                    nc.vector.tensor_copy(hT_sb[:, :rw], ps_tr[:, :rw])
                    if layer == 0:
                        nc.sync.dma_start(out=h1T[:, r0:r0 + rw], in_=hT_sb[:, :rw])
                    else:
                        # decoder: recon = h2 @ Wd
                        ps_r = psepi.tile([P, HID], f32, tag="psr")
                        nc.tensor.matmul(ps_r[:rw, :], lhsT=hT_sb[:, :rw],
                                         rhs=wd_sb[:], start=True, stop=True)
                        rrow = epipool.tile([P, HID], f32, tag="rrow")
                        nc.vector.tensor_copy(rrow[:rw, :], ps_r[:rw, :])
                        nc.sync.dma_start(out=recon[r0:r0 + rw, :], in_=rrow[:rw, :])

            def phase_enabled():
                order = ["dense0", "ag0", "edge0", "dense1", "ag1", "edge1",
                         "full"]
                return order.index(stop_after)

            lim = phase_enabled()
            # ---------------- layer 0
            dense_phase(0, xT)
            if lim >= 1:
                nc.gpsimd.collective_compute(
                    "AllGather", mybir.AluOpType.bypass, replica_groups=RG,
                    ins=[fext_shard[:, :]], outs=[fext_full[:, :]])
            if lim >= 2:
                edge_phase(0)
            if lim >= 3:
                dense_phase(1, h1T)
            if lim >= 4:
                nc.gpsimd.collective_compute(
                    "AllGather", mybir.AluOpType.bypass, replica_groups=RG,
                    ins=[fext_shard[:, :]], outs=[fext_full[:, :]])
            if lim >= 5:
                edge_phase(1)

            # ---------------- masked MSE partials
            macc = cpool.tile([P, 1], f32, tag="macc")
            nc.vector.memset(macc[:], 0.0)
            midx = cpool.tile([P, TM], i32, tag="midx")
            nc.sync.dma_start(out=midx[:], in_=maskpk[:])
            mval = cpool.tile([P, TM], f32, tag="mval")
            nc.sync.dma_start(out=mval[:], in_=maskval[:])
            for m in range(TM if lim >= 6 else 0):
                rg = msepool.tile([P, IN_DIM], f32, tag="rg")
                nc.gpsimd.indirect_dma_start(
                    out=rg[:], out_offset=None, in_=recon[:, :],
                    in_offset=IOA(ap=midx[:, m:m + 1], axis=0))
                am = msepool.tile([P, IN_DIM], f32, tag="am")
                nc.sync.dma_start(out=am[:], in_=amask[m * P:(m + 1) * P, :])
                diff = msepool.tile([P, IN_DIM], f32, tag="diff")
                # (recon * val) - amask ; amask pad rows are 0
                nc.vector.scalar_tensor_tensor(
                    out=diff[:], in0=rg[:], scalar=mval[:, m:m + 1], in1=am[:],
                    op0=OP.mult, op1=OP.subtract)
                sq = msepool.tile([P, IN_DIM], f32, tag="sq")
                nc.vector.tensor_tensor(out=sq[:], in0=diff[:], in1=diff[:],
                                        op=OP.mult)
                pacc = msepool.tile([P, 1], f32, tag="pacc")
                nc.vector.tensor_reduce(out=pacc[:], in_=sq[:],
                                        axis=mybir.AxisListType.XYZW, op=OP.add)
                nc.vector.tensor_add(macc[:], macc[:], pacc[:])
            nc.sync.dma_start(out=msepart[:], in_=macc[:])

    nc.compile()
    return nc


def _prep_inputs(attr, mask_idx, W0, al0, ar0, b0, W1, al1, ar1, b1, Wd, bd,
                 mask_token, g):
    attr = np.asarray(attr, np.float32)
    W0 = np.asarray(W0, np.float32); W1 = np.asarray(W1, np.float32)
    al0 = np.asarray(al0, np.float32); ar0 = np.asarray(ar0, np.float32)
    al1 = np.asarray(al1, np.float32); ar1 = np.asarray(ar1, np.float32)
    Wd32 = np.asarray(Wd, np.float32)
    b0 = np.asarray(b0, np.float32); b1 = np.asarray(b1, np.float32)
    bd32 = np.asarray(bd, np.float32)

    def wext(W, al, ar):
        Wr = W.reshape(IN_DIM, HEADS, D)
        wal = np.einsum("khd,hd->kh", Wr, al)
        war = np.einsum("khd,hd->kh", Wr, ar)
        return np.ascontiguousarray(
            np.concatenate([W, wal, war], axis=1).astype(np.float32))

    wext0 = wext(W0, al0, ar0)
    wext1 = wext(W1, al1, ar1)
    brep0 = np.ascontiguousarray(np.tile(b0[None, :], (P, 1)))
    brep1 = np.ascontiguousarray(np.tile(b1[None, :], (P, 1)))
    iota = np.ascontiguousarray(
        np.tile(np.arange(P, dtype=np.float32)[None, :], (P, 1)))

    attr_m = attr.copy()
    attr_m[np.asarray(mask_idx).astype(np.int64)] = np.asarray(mask_token, np.float32)
    attr_mT = np.ascontiguousarray(attr_m.T)      # [128, N]

    TM = g["TM"]
    sel = g["mask_rows_sel"]                       # [NCORES, TM*P] global idx
    val = g["maskval_flat"]                        # [NCORES, TM*P]
    rows = attr[sel.reshape(-1)].reshape(NCORES, TM * P, IN_DIM)
    amask = (rows - bd32[None, None, :]) * val[:, :, None]
    amask = np.ascontiguousarray(amask.astype(np.float32))

    in_maps = []
    for c in range(NCORES):
        in_maps.append({
            "xt": np.ascontiguousarray(attr_mT[:, c * ROWS:(c + 1) * ROWS]),
            "wext0": wext0, "wext1": wext1,
            "wd": np.ascontiguousarray(Wd32),
            "brep0": brep0, "brep1": brep1,
            "iota": iota,
            "srcpk": g["srcpk"][c], "dstpk": g["dstpk"][c],
            "maskpk": g["maskpk"][c], "maskval": g["maskval"][c],
            "amask": amask[c],
        })
    return in_maps


# ------------------------------------------------------------------ runner

def _get_runner(nc):
    """Cached jitted SPMD executor (run_bass_via_pjrt with a persistent jit)."""
    import jax
    import jax.numpy as jnp
    from jax.sharding import Mesh, PartitionSpec
    from jax.experimental.shard_map import shard_map
    from concourse import bass2jax, mybir
    from concourse.bass2jax import _bass_exec_p, partition_id_tensor

    bass2jax.install_neuronx_cc_hook()

    partition_name = (nc.partition_id_tensor.name
                      if nc.partition_id_tensor else None)
    in_names, out_names, out_avals, zero_outs = [], [], [], []
    for alloc in nc.m.functions[0].allocations:
        if not isinstance(alloc, mybir.MemoryLocationSet):
            continue
        name = alloc.memorylocations[0].name
        if alloc.kind == "ExternalInput":
            if name != partition_name:
                in_names.append(name)
        elif alloc.kind == "ExternalOutput":
            shape = tuple(alloc.tensor_shape)
            dtype = mybir.dt.np(alloc.dtype)
            out_names.append(name)
            out_avals.append(jax.core.ShapedArray(shape, dtype))
            zero_outs.append(np.zeros(shape, dtype))
    n_params = len(in_names)
    n_outs = len(out_avals)
    all_in_names = list(in_names) + list(out_names)
    if partition_name is not None:
        all_in_names.append(partition_name)

    def _body(*args):
        operands = list(args)
        if partition_name is not None:
            operands.append(partition_id_tensor())
        outs = _bass_exec_p.bind(
            *operands,
            out_avals=tuple(out_avals),
            in_names=tuple(all_in_names),
            out_names=tuple(out_names),
            lowering_input_output_aliases=(),
            sim_require_finite=True,
            sim_require_nnan=True,
            nc=nc,
        )
        return tuple(outs)

    devices = jax.devices()[:NCORES]
    mesh = Mesh(np.asarray(devices), ("core",))
    in_specs = (PartitionSpec("core"),) * (n_params + n_outs)
    out_specs = (PartitionSpec("core"),) * n_outs
    donate = tuple(range(n_params, n_params + n_outs))
    fn = jax.jit(
        shard_map(_body, mesh=mesh, in_specs=in_specs, out_specs=out_specs,
                  check_rep=False),
        donate_argnums=donate, keep_unused=True)

    from jax.sharding import NamedSharding
    shard = NamedSharding(mesh, PartitionSpec("core"))
    dev_cache = {}

    def dispatch(in_maps):
        key = id(in_maps)
        if dev_cache.get("key") != key:
            per_core = [[np.asarray(m[n]) for n in in_names] for m in in_maps]
            concat_in = [
                np.concatenate([per_core[c][i] for c in range(NCORES)], axis=0)
                for i in range(n_params)]
            dev_cache["in"] = [jax.device_put(a, shard) for a in concat_in]
            dev_cache["key"] = key
        concat_zeros = [np.zeros((NCORES * z.shape[0], *z.shape[1:]), z.dtype)
                        for z in zero_outs]
        return fn(*dev_cache["in"], *concat_zeros)

    def finalize(out_arrs):
        return [
            {name: np.asarray(out_arrs[i]).reshape(NCORES, *out_avals[i].shape)[c]
             for i, name in enumerate(out_names)}
            for c in range(NCORES)
        ]

    def run(in_maps):
        return finalize(dispatch(in_maps))

    run.dispatch = dispatch
    run.finalize = finalize
    run.dev_cache = dev_cache
    return run


# ------------------------------------------------------------------- kernel

def _reduce_out(results, nmask):
    total = sum(float(np.asarray(r["msepart"]).sum()) for r in results)
    return np.float32(total / (nmask * IN_DIM))


def kernel(attr, src, dst, mask_idx, W0, al0, ar0, b0, W1, al1, ar1, b1, Wd, bd, mask_token):
    attr = np.asarray(attr)
    src_r = np.asarray(src)
    dst_r = np.asarray(dst)
    mask_r = np.asarray(mask_idx)

    def fps():
        gfp = _fingerprint(src_r, dst_r, mask_r)
        afp = _fingerprint(attr, np.asarray(W0), np.asarray(W1), np.asarray(Wd),
                           np.asarray(al0), np.asarray(ar0), np.asarray(al1),
                           np.asarray(ar1), np.asarray(b0), np.asarray(b1),
                           np.asarray(bd), np.asarray(mask_token))
        return gfp, afp

    # warm path: dispatch asynchronously with cached device inputs, verify
    # input fingerprints while the device runs, fall back on mismatch
    run = _CACHE.get("run")
    if run is not None and "in_maps" in _CACHE and \
            run.dev_cache.get("key") == id(_CACHE["in_maps"]):
        out_arrs = run.dispatch(_CACHE["in_maps"])
        gfp, afp = fps()
        if gfp == _CACHE.get("gfp") and afp == _CACHE.get("afp"):
            return _reduce_out(run.finalize(out_arrs), mask_r.shape[0])
    else:
        gfp, afp = fps()

    # cold / changed-input path
    attr32 = np.asarray(attr, dtype=np.float32)
    src64 = src_r.astype(np.int64)
    dst64 = dst_r.astype(np.int64)
    mask64 = mask_r.astype(np.int64)

    if _CACHE.get("gfp") != gfp or "graph" not in _CACHE:
        _CACHE["gfp"] = gfp
        _CACHE["graph"] = _prep_graph(src64, dst64, mask64)
        _CACHE.pop("afp", None)
    g = _CACHE["graph"]

    key = (g["T"], g["TM"])
    if _CACHE.get("progkey") != key:
        nc = _build_program(list(g["T"]), g["TT"], g["TM"])
        _CACHE["run"] = _get_runner(nc)
        _CACHE["progkey"] = key

    if _CACHE.get("afp") != afp or "in_maps" not in _CACHE:
        _CACHE["afp"] = afp
        _CACHE["in_maps"] = _prep_inputs(
            attr32, mask64, W0, al0, ar0, b0, W1, al1, ar1, b1, Wd, bd,
            mask_token, g)

    return _reduce_out(_CACHE["run"](_CACHE["in_maps"]), mask_r.shape[0])


# revision 5
# speedup vs baseline: 1.4942x; 1.4942x over previous
"""2-layer GAT masked-autoencoder MSE, fully on 8 Trainium2 cores.

Single Bass/Tile SPMD program:
  - nodes sharded by range (12500/core); edges assigned to the core
    owning their dst, sorted by dst, grouped into 128-dst-node blocks
  - per layer: dense matmul h@[W|wal|war] -> per-node (f, el, er);
    AllGather of the [N,136] feature table; per 128-edge tile an
    indirect-DMA gather of src rows + one-hot selection matmul does
    edge softmax numerators and denominators in one pass
  - decoder matmul + masked-MSE partial sums fused into layer-2 epilogue
Host only packs index tensors (cached by content fingerprint) and
averages the 8x128 partial sums.
"""

import sys
import hashlib
import math
import numpy as np

for _p in ("/opt/trn_rl_repo", "/root/.axon_site/_ro/trn_rl_repo"):
    if _p not in sys.path:
        sys.path.append(_p)

N = 100000
E = 1600000
IN_DIM = 128
HID = 128
HEADS = 4
D = HID // HEADS
NEG_SLOPE = 0.2
NCORES = 8
ROWS = N // NCORES          # 12500
P = 128
NBLK = (ROWS + P - 1) // P  # 98
FE = HID + 2 * HEADS        # 136: [f(128) | el(4) | er(4)]
MASK_N = 30000

_CACHE = {}

# v2 edge-phase feature flags (bisection)
# note: multi-row indirect gather (2 rows/partition/DMA) passes CoreSim but
# returns garbage on hardware -- do not use.
V2_BD = True      # S_T via broadcast-DMA instead of PE transpose
V2_PSACC = True   # PSUM accumulation across a block's tiles
V2_FW = True      # fused 4-head multiply via 3D broadcast AP


def _fingerprint(*arrs):
    import zlib
    parts = []
    for a in arrs:
        a = np.ascontiguousarray(a)
        parts.append((a.shape, str(a.dtype),
                      zlib.crc32(memoryview(a.reshape(-1).view(np.uint8)))))
    return tuple(parts)


# ----------------------------------------------------------------- host prep

def _prep_graph(src, dst, mask_idx):
    """Pack edges into per-core [128, TT] tiles grouped by dst block."""
    core_of = dst // ROWS
    loc = dst - core_of * ROWS
    blk = loc >> 7
    rel = (loc & 127).astype(np.float32)

    order = np.argsort(dst, kind="stable")
    src_s = src[order].astype(np.int32)
    core_s = core_of[order]
    blk_s = blk[order]
    rel_s = rel[order]

    flat = (core_s * NBLK + blk_s).astype(np.int64)
    gcounts = np.bincount(flat, minlength=NCORES * NBLK)
    # tiles per block: max over cores, >=1
    cnt_cb = gcounts.reshape(NCORES, NBLK)
    T = np.maximum(1, (cnt_cb.max(axis=0) + P - 1) // P).astype(np.int64)
    TT = int(T.sum())
    tile_base = np.zeros(NBLK, np.int64)
    tile_base[1:] = np.cumsum(T)[:-1]

    gstart = np.zeros(NCORES * NBLK + 1, np.int64)
    gstart[1:] = np.cumsum(gcounts)
    k = np.arange(src_s.shape[0], dtype=np.int64) - gstart[flat]
    pos = tile_base[blk_s] * P + k            # slot within core's packed array

    srcpk = np.zeros((NCORES, TT * P), np.int32)
    dstpk = np.full((NCORES, TT * P), 999.0, np.float32)
    srcpk[core_s, pos] = src_s
    dstpk[core_s, pos] = rel_s
    # [TT, 128] row-major copy (row t = tile t's edges) for broadcast DMA
    dstrow = np.ascontiguousarray(dstpk.reshape(NCORES, TT, P))
    # [TT*P] -> [128, TT] (partition p = edge p of tile t)
    srcpk = np.ascontiguousarray(
        srcpk.reshape(NCORES, TT, P).transpose(0, 2, 1))
    dstpk = np.ascontiguousarray(
        dstpk.reshape(NCORES, TT, P).transpose(0, 2, 1))

    # masks
    m_core = (mask_idx // ROWS).astype(np.int64)
    m_loc = (mask_idx - m_core * ROWS).astype(np.int32)
    mc = np.bincount(m_core, minlength=NCORES)
    TM = int((mc.max() + P - 1) // P)
    maskpk = np.zeros((NCORES, TM * P), np.int32)
    maskval = np.zeros((NCORES, TM * P), np.float32)
    mask_rows_sel = np.zeros((NCORES, TM * P), np.int64)  # global idx for attr row fetch
    morder = np.argsort(m_core, kind="stable")
    ms_core = m_core[morder]
    ms_loc = m_loc[morder]
    ms_glob = mask_idx[morder].astype(np.int64)
    mstart = np.zeros(NCORES + 1, np.int64)
    mstart[1:] = np.cumsum(mc)
    mk = np.arange(mask_idx.shape[0], dtype=np.int64) - mstart[ms_core]
    maskpk[ms_core, mk] = ms_loc
    maskval[ms_core, mk] = 1.0
    mask_rows_sel[ms_core, mk] = ms_glob
    maskval_flat = maskval.copy()
    maskpk = np.ascontiguousarray(maskpk.reshape(NCORES, TM, P).transpose(0, 2, 1))
    maskval = np.ascontiguousarray(maskval.reshape(NCORES, TM, P).transpose(0, 2, 1))

    return dict(T=tuple(int(x) for x in T), TT=TT, TM=TM,
                srcpk=srcpk, dstpk=dstpk, dstrow=dstrow,
                maskpk=maskpk, maskval=maskval, mask_rows_sel=mask_rows_sel,
                maskval_flat=maskval_flat)


# ------------------------------------------------------------- program build

def _build_program(T, TT, TM, ncores=None, stop_after="full"):
    if ncores is None:
        ncores = NCORES
    from concourse import bacc, bass, mybir
    from concourse import tile
    from concourse.masks import make_identity

    f32 = mybir.dt.float32
    i32 = mybir.dt.int32
    IOA = bass.IndirectOffsetOnAxis
    OP = mybir.AluOpType
    ACT = mybir.ActivationFunctionType
    T_MAX = max(T)

    nc = bacc.Bacc()

    # ---- I/O
    xT = nc.declare_dram_parameter("xt", [P, ROWS], f32, isOutput=False)
    wext0 = nc.declare_dram_parameter("wext0", [P, FE], f32, isOutput=False)
    wext1 = nc.declare_dram_parameter("wext1", [P, FE], f32, isOutput=False)
    wd = nc.declare_dram_parameter("wd", [P, HID], f32, isOutput=False)
    brep0 = nc.declare_dram_parameter("brep0", [P, HID], f32, isOutput=False)
    brep1 = nc.declare_dram_parameter("brep1", [P, HID], f32, isOutput=False)
    iota_in = nc.declare_dram_parameter("iota", [P, P], f32, isOutput=False)
    iotac_in = nc.declare_dram_parameter("iotac", [P, 1], f32, isOutput=False)
    srcpk = nc.declare_dram_parameter("srcpk", [P, TT], i32, isOutput=False)
    dstpk = nc.declare_dram_parameter("dstpk", [P, TT], f32, isOutput=False)
    dstrow = nc.declare_dram_parameter("dstrow", [TT, P], f32, isOutput=False)
    maskpk = nc.declare_dram_parameter("maskpk", [P, TM], i32, isOutput=False)
    maskval = nc.declare_dram_parameter("maskval", [P, TM], f32, isOutput=False)
    amask = nc.declare_dram_parameter("amask", [TM * P, IN_DIM], f32, isOutput=False)
    msepart = nc.declare_dram_parameter("msepart", [P, 1], f32, isOutput=True)

    # ---- DRAM scratch
    fext_shard = nc.dram_tensor("fext_shard", [ROWS, FE], f32, kind="Internal")
    fext_full = nc.dram_tensor("fext_full", [N, FE], f32, kind="Internal",
                               addr_space="Shared")
    h1T = nc.dram_tensor("h1T", [P, ROWS], f32, kind="Internal")
    recon = nc.dram_tensor("recon", [ROWS, HID], f32, kind="Internal")

    RG = [list(range(ncores))]

    with tile.TileContext(nc) as tc:
        with (
            tc.tile_pool(name="const", bufs=1) as cpool,
            tc.tile_pool(name="wrow", bufs=1) as wpool,
            tc.tile_pool(name="xin", bufs=3) as xpool,
            tc.tile_pool(name="fout", bufs=3) as fpool,
            tc.tile_pool(name="eres", bufs=1) as erpool,
            tc.tile_pool(name="blkidx", bufs=2) as bipool,
            tc.tile_pool(name="fx", bufs=T_MAX + 3) as fxpool,
            tc.tile_pool(name="smat", bufs=T_MAX + 3) as spool,
            tc.tile_pool(name="stmat", bufs=4) as stpool,
            tc.tile_pool(name="estg", bufs=2) as epool,
            tc.tile_pool(name="rhs", bufs=3) as rhspool,
            tc.tile_pool(name="epi", bufs=3) as epipool,
            tc.tile_pool(name="mse", bufs=3) as msepool,
            tc.tile_pool(name="ps_t", bufs=2, space="PSUM") as pst,
            tc.tile_pool(name="ps_er", bufs=2, space="PSUM") as pser,
            tc.tile_pool(name="ps_mm", bufs=2, space="PSUM") as psmm,
            tc.tile_pool(name="ps_epi", bufs=2, space="PSUM") as psepi,
        ):
            ident = cpool.tile([P, P], f32, tag="ident")
            make_identity(nc, ident[:])
            iota_sb = cpool.tile([P, P], f32, tag="iota")
            nc.sync.dma_start(out=iota_sb[:], in_=iota_in[:])
            iotac_sb = cpool.tile([P, 1], f32, tag="iotac")
            nc.sync.dma_start(out=iotac_sb[:], in_=iotac_in[:])
            wd_sb = cpool.tile([P, HID], f32, tag="wd")
            nc.sync.dma_start(out=wd_sb[:], in_=wd[:])
            brep_sb = [cpool.tile([P, HID], f32, tag=f"brep{l}", name=f"brep_sb{l}")
                       for l in range(2)]
            nc.sync.dma_start(out=brep_sb[0][:], in_=brep0[:])
            nc.sync.dma_start(out=brep_sb[1][:], in_=brep1[:])
            wext_sb = [cpool.tile([P, FE], f32, tag=f"wext{l}", name=f"wext_sb{l}")
                       for l in range(2)]
            nc.sync.dma_start(out=wext_sb[0][:], in_=wext0[:])
            nc.sync.dma_start(out=wext_sb[1][:], in_=wext1[:])

            er_res = erpool.tile([P, HEADS * NBLK], f32, tag="er_res")

            def dense_phase(layer, lhsT_dram):
                """f_ext = h @ Wext; writes fext_shard + er_res."""
                nc.vector.memset(er_res[:], 0.0)
                for b in range(NBLK):
                    r0 = b * P
                    rw = min(P, ROWS - r0)
                    xt_t = xpool.tile([P, P], f32, tag="xt")
                    nc.sync.dma_start(out=xt_t[:, :rw], in_=lhsT_dram[:, r0:r0 + rw])
                    ps = psmm.tile([P, FE], f32, tag="mm")
                    nc.tensor.matmul(ps[:rw, :], lhsT=xt_t[:, :rw], rhs=wext_sb[layer][:],
                                     start=True, stop=True)
                    f_t = fpool.tile([P, FE], f32, tag="f")
                    nc.vector.tensor_copy(f_t[:rw, :], ps[:rw, :])
                    nc.vector.tensor_copy(er_res[:rw, HEADS * b:HEADS * (b + 1)],
                                          f_t[:rw, HID + HEADS:HID + 2 * HEADS])
                    nc.sync.dma_start(out=fext_shard[r0:r0 + rw, :], in_=f_t[:rw, :])

            def edge_phase(layer):
                tglob = 0
                for b in range(NBLK):
                    r0 = b * P
                    rw = min(P, ROWS - r0)
                    Tb = T[b]
                    sidx = bipool.tile([P, T_MAX], i32, tag="sidx")
                    nc.sync.dma_start(out=sidx[:, :Tb], in_=srcpk[:, tglob:tglob + Tb])
                    drel = bipool.tile([P, T_MAX], f32, tag="drel")
                    nc.sync.dma_start(out=drel[:, :Tb], in_=dstpk[:, tglob:tglob + Tb])

                    estage = epool.tile([P, HEADS * T_MAX], f32, tag="estage")
                    fx_list, s_list = [], []
                    for t in range(Tb):
                        gt = fxpool.tile([P, FE], f32, tag="fx")
                        nc.gpsimd.indirect_dma_start(
                            out=gt[:], out_offset=None,
                            in_=fext_full[:, :],
                            in_offset=IOA(ap=sidx[:, t:t + 1], axis=0))
                        fx_list.append((gt, 0))
                    for t in range(Tb):
                        gt_t, base = fx_list[t]
                        S = spool.tile([P, P], f32, tag="S")
                        nc.vector.tensor_tensor(
                            out=S[:], in0=drel[:, t:t + 1].to_broadcast([P, P]),
                            in1=iota_sb[:], op=OP.is_equal)
                        if V2_BD:
                            dstrep = stpool.tile([P, P], f32, tag="dstrep")
                            nc.sync.dma_start(
                                out=dstrep[:],
                                in_=dstrow[tglob + t:tglob + t + 1, :]
                                .to_broadcast([P, P]))
                            sT = stpool.tile([P, P], f32, tag="sT")
                            nc.vector.tensor_tensor(
                                out=sT[:], in0=iotac_sb[:, 0:1].to_broadcast([P, P]),
                                in1=dstrep[:], op=OP.is_equal)
                        else:
                            ps_t_ = pst.tile([P, P], f32, tag="pst")
                            nc.tensor.transpose(ps_t_[:], S[:], ident[:])
                            sT = stpool.tile([P, P], f32, tag="sT")
                            nc.vector.tensor_copy(sT[:], ps_t_[:])
                        ps_e = pser.tile([P, HEADS], f32, tag="pser")
                        nc.tensor.matmul(ps_e[:], lhsT=sT[:],
                                         rhs=er_res[:, HEADS * b:HEADS * (b + 1)],
                                         start=True, stop=True)
                        nc.vector.tensor_tensor(
                            out=estage[:, HEADS * t:HEADS * (t + 1)],
                            in0=gt_t[:, base + HID:base + HID + HEADS],
                            in1=ps_e[:], op=OP.add)
                        s_list.append(S)
                    # leaky-relu + exp for the whole block in 2 ops
                    nc.vector.scalar_tensor_tensor(
                        out=estage[:, :HEADS * Tb], in0=estage[:, :HEADS * Tb],
                        scalar=NEG_SLOPE, in1=estage[:, :HEADS * Tb],
                        op0=OP.mult, op1=OP.max)
                    exstage = epool.tile([P, HEADS * T_MAX], f32, tag="exstage")
                    nc.scalar.activation(exstage[:, :HEADS * Tb],
                                         estage[:, :HEADS * Tb], ACT.Exp)

                    acc_ps = (psmm.tile([P, FE], f32, tag="mm", name="acc_ps")
                              if V2_PSACC else None)
                    acc_sb = None
                    for t in range(Tb):
                        gt_t, base = fx_list[t]
                        rhs = rhspool.tile([P, FE - HEADS], f32, tag="rhs")
                        if V2_FW:
                            nc.vector.tensor_tensor(
                                out=rhs[:, 0:HID].rearrange("p (h d) -> p h d",
                                                            h=HEADS),
                                in0=gt_t[:, base:base + HID].rearrange(
                                    "p (h d) -> p h d", h=HEADS),
                                in1=exstage[:, HEADS * t:HEADS * (t + 1)]
                                .to_broadcast([P, HEADS, D]),
                                op=OP.mult)
                        else:
                            for h in range(HEADS):
                                nc.vector.tensor_scalar(
                                    rhs[:, D * h:D * (h + 1)],
                                    gt_t[:, base + D * h:base + D * (h + 1)],
                                    exstage[:, HEADS * t + h:HEADS * t + h + 1],
                                    None, OP.mult)
                        nc.vector.tensor_copy(rhs[:, HID:HID + HEADS],
                                              exstage[:, HEADS * t:HEADS * (t + 1)])
                        if V2_PSACC:
                            nc.tensor.matmul(acc_ps[:, :FE - HEADS],
                                             lhsT=s_list[t][:],
                                             rhs=rhs[:], start=(t == 0),
                                             stop=(t == Tb - 1),
                                             skip_group_check=True)
                        else:
                            ps = psmm.tile([P, FE], f32, tag="mm")
                            nc.tensor.matmul(ps[:, :FE - HEADS], lhsT=s_list[t][:],
                                             rhs=rhs[:], start=True, stop=True)
                            if acc_sb is None:
                                acc_sb = epipool.tile([P, FE - HEADS], f32,
                                                      tag="accsb")
                                nc.vector.tensor_copy(acc_sb[:], ps[:, :FE - HEADS])
                            else:
                                nc.vector.tensor_add(acc_sb[:], acc_sb[:],
                                                     ps[:, :FE - HEADS])
                    tglob += Tb
                    acc_r = acc_ps if V2_PSACC else acc_sb

                    # epilogue: h = relu(num/den + b)
                    rec = epipool.tile([P, HEADS], f32, tag="rec")
                    nc.vector.tensor_scalar(rec[:], acc_r[:, HID:HID + HEADS],
                                            1e-20, None, OP.add)
                    nc.vector.reciprocal(rec[:], rec[:])
                    hrow = epipool.tile([P, HID], f32, tag="hrow")
                    for h in range(HEADS):
                        nc.vector.scalar_tensor_tensor(
                            out=hrow[:, D * h:D * (h + 1)],
                            in0=acc_r[:, D * h:D * (h + 1)],
                            scalar=rec[:, h:h + 1],
                            in1=brep_sb[layer][:, D * h:D * (h + 1)],
                            op0=OP.mult, op1=OP.add)
                    hrow2 = epipool.tile([P, HID], f32, tag="hrow2")
                    nc.scalar.activation(hrow2[:], hrow[:], ACT.Relu)
                    # transpose h rows -> [feat, node]
                    ps_tr = pst.tile([P, P], f32, tag="pst")
                    nc.tensor.transpose(ps_tr[:, :rw], hrow2[:rw, :],
                                        ident[:rw, :rw])
                    hT_sb = epipool.tile([P, P], f32, tag="hT")
                    nc.vector.tensor_copy(hT_sb[:, :rw], ps_tr[:, :rw])
                    if layer == 0:
                        nc.sync.dma_start(out=h1T[:, r0:r0 + rw], in_=hT_sb[:, :rw])
                    else:
                        # decoder: recon = h2 @ Wd
                        ps_r = psepi.tile([P, HID], f32, tag="psr")
                        nc.tensor.matmul(ps_r[:rw, :], lhsT=hT_sb[:, :rw],
                                         rhs=wd_sb[:], start=True, stop=True)
                        rrow = epipool.tile([P, HID], f32, tag="rrow")
                        nc.vector.tensor_copy(rrow[:rw, :], ps_r[:rw, :])
                        nc.sync.dma_start(out=recon[r0:r0 + rw, :], in_=rrow[:rw, :])

            def phase_enabled():
                order = ["dense0", "ag0", "edge0", "dense1", "ag1", "edge1",
                         "full"]
                return order.index(stop_after)

            lim = phase_enabled()
            # ---------------- layer 0
            dense_phase(0, xT)
            if lim >= 1:
                nc.gpsimd.collective_compute(
                    "AllGather", mybir.AluOpType.bypass, replica_groups=RG,
                    ins=[fext_shard[:, :]], outs=[fext_full[:, :]])
            if lim >= 2:
                edge_phase(0)
            if lim >= 3:
                dense_phase(1, h1T)
            if lim >= 4:
                nc.gpsimd.collective_compute(
                    "AllGather", mybir.AluOpType.bypass, replica_groups=RG,
                    ins=[fext_shard[:, :]], outs=[fext_full[:, :]])
            if lim >= 5:
                edge_phase(1)

            # ---------------- masked MSE partials
            macc = cpool.tile([P, 1], f32, tag="macc")
            nc.vector.memset(macc[:], 0.0)
            midx = cpool.tile([P, TM], i32, tag="midx")
            nc.sync.dma_start(out=midx[:], in_=maskpk[:])
            mval = cpool.tile([P, TM], f32, tag="mval")
            nc.sync.dma_start(out=mval[:], in_=maskval[:])
            for m in range(TM if lim >= 6 else 0):
                rg = msepool.tile([P, IN_DIM], f32, tag="rg")
                nc.gpsimd.indirect_dma_start(
                    out=rg[:], out_offset=None, in_=recon[:, :],
                    in_offset=IOA(ap=midx[:, m:m + 1], axis=0))
                am = msepool.tile([P, IN_DIM], f32, tag="am")
                nc.sync.dma_start(out=am[:], in_=amask[m * P:(m + 1) * P, :])
                diff = msepool.tile([P, IN_DIM], f32, tag="diff")
                # (recon * val) - amask ; amask pad rows are 0
                nc.vector.scalar_tensor_tensor(
                    out=diff[:], in0=rg[:], scalar=mval[:, m:m + 1], in1=am[:],
                    op0=OP.mult, op1=OP.subtract)
                sq = msepool.tile([P, IN_DIM], f32, tag="sq")
                nc.vector.tensor_tensor(out=sq[:], in0=diff[:], in1=diff[:],
                                        op=OP.mult)
                pacc = msepool.tile([P, 1], f32, tag="pacc")
                nc.vector.tensor_reduce(out=pacc[:], in_=sq[:],
                                        axis=mybir.AxisListType.X, op=OP.add)
                nc.vector.tensor_add(macc[:], macc[:], pacc[:])
            nc.sync.dma_start(out=msepart[:], in_=macc[:])

    nc.compile()
    return nc


def _prep_inputs(attr, mask_idx, W0, al0, ar0, b0, W1, al1, ar1, b1, Wd, bd,
                 mask_token, g):
    attr = np.asarray(attr, np.float32)
    W0 = np.asarray(W0, np.float32); W1 = np.asarray(W1, np.float32)
    al0 = np.asarray(al0, np.float32); ar0 = np.asarray(ar0, np.float32)
    al1 = np.asarray(al1, np.float32); ar1 = np.asarray(ar1, np.float32)
    Wd32 = np.asarray(Wd, np.float32)
    b0 = np.asarray(b0, np.float32); b1 = np.asarray(b1, np.float32)
    bd32 = np.asarray(bd, np.float32)

    def wext(W, al, ar):
        Wr = W.reshape(IN_DIM, HEADS, D)
        wal = np.einsum("khd,hd->kh", Wr, al)
        war = np.einsum("khd,hd->kh", Wr, ar)
        return np.ascontiguousarray(
            np.concatenate([W, wal, war], axis=1).astype(np.float32))

    wext0 = wext(W0, al0, ar0)
    wext1 = wext(W1, al1, ar1)
    brep0 = np.ascontiguousarray(np.tile(b0[None, :], (P, 1)))
    brep1 = np.ascontiguousarray(np.tile(b1[None, :], (P, 1)))
    iota = np.ascontiguousarray(
        np.tile(np.arange(P, dtype=np.float32)[None, :], (P, 1)))
    iotac = np.ascontiguousarray(np.arange(P, dtype=np.float32)[:, None])

    attr_m = attr.copy()
    attr_m[np.asarray(mask_idx).astype(np.int64)] = np.asarray(mask_token, np.float32)
    attr_mT = np.ascontiguousarray(attr_m.T)      # [128, N]

    TM = g["TM"]
    sel = g["mask_rows_sel"]                       # [NCORES, TM*P] global idx
    val = g["maskval_flat"]                        # [NCORES, TM*P]
    rows = attr[sel.reshape(-1)].reshape(NCORES, TM * P, IN_DIM)
    amask = (rows - bd32[None, None, :]) * val[:, :, None]
    amask = np.ascontiguousarray(amask.astype(np.float32))

    in_maps = []
    for c in range(NCORES):
        in_maps.append({
            "xt": np.ascontiguousarray(attr_mT[:, c * ROWS:(c + 1) * ROWS]),
            "wext0": wext0, "wext1": wext1,
            "wd": np.ascontiguousarray(Wd32),
            "brep0": brep0, "brep1": brep1,
            "iota": iota, "iotac": iotac,
            "srcpk": g["srcpk"][c], "dstpk": g["dstpk"][c],
            "dstrow": g["dstrow"][c],
            "maskpk": g["maskpk"][c], "maskval": g["maskval"][c],
            "amask": amask[c],
        })
    return in_maps


# ------------------------------------------------------------------ runner

def _get_runner(nc):
    """Cached jitted SPMD executor (run_bass_via_pjrt with a persistent jit)."""
    import jax
    import jax.numpy as jnp
    from jax.sharding import Mesh, PartitionSpec
    from jax.experimental.shard_map import shard_map
    from concourse import bass2jax, mybir
    from concourse.bass2jax import _bass_exec_p, partition_id_tensor

    bass2jax.install_neuronx_cc_hook()

    partition_name = (nc.partition_id_tensor.name
                      if nc.partition_id_tensor else None)
    in_names, out_names, out_avals, zero_outs = [], [], [], []
    for alloc in nc.m.functions[0].allocations:
        if not isinstance(alloc, mybir.MemoryLocationSet):
            continue
        name = alloc.memorylocations[0].name
        if alloc.kind == "ExternalInput":
            if name != partition_name:
                in_names.append(name)
        elif alloc.kind == "ExternalOutput":
            shape = tuple(alloc.tensor_shape)
            dtype = mybir.dt.np(alloc.dtype)
            out_names.append(name)
            out_avals.append(jax.core.ShapedArray(shape, dtype))
            zero_outs.append(np.zeros(shape, dtype))
    n_params = len(in_names)
    n_outs = len(out_avals)
    all_in_names = list(in_names) + list(out_names)
    if partition_name is not None:
        all_in_names.append(partition_name)

    def _body(*args):
        operands = list(args)
        if partition_name is not None:
            operands.append(partition_id_tensor())
        outs = _bass_exec_p.bind(
            *operands,
            out_avals=tuple(out_avals),
            in_names=tuple(all_in_names),
            out_names=tuple(out_names),
            lowering_input_output_aliases=(),
            sim_require_finite=True,
            sim_require_nnan=True,
            nc=nc,
        )
        return tuple(outs)

    devices = jax.devices()[:NCORES]
    mesh = Mesh(np.asarray(devices), ("core",))
    in_specs = (PartitionSpec("core"),) * (n_params + n_outs)
    out_specs = (PartitionSpec("core"),) * n_outs
    donate = tuple(range(n_params, n_params + n_outs))
    fn = jax.jit(
        shard_map(_body, mesh=mesh, in_specs=in_specs, out_specs=out_specs,
                  check_rep=False),
        donate_argnums=donate, keep_unused=True)

    from jax.sharding import NamedSharding
    shard = NamedSharding(mesh, PartitionSpec("core"))
    dev_cache = {}

    def dispatch(in_maps):
        key = id(in_maps)
        if dev_cache.get("key") != key:
            per_core = [[np.asarray(m[n]) for n in in_names] for m in in_maps]
            concat_in = [
                np.concatenate([per_core[c][i] for c in range(NCORES)], axis=0)
                for i in range(n_params)]
            dev_cache["in"] = [jax.device_put(a, shard) for a in concat_in]
            dev_cache["key"] = key
        concat_zeros = [np.zeros((NCORES * z.shape[0], *z.shape[1:]), z.dtype)
                        for z in zero_outs]
        return fn(*dev_cache["in"], *concat_zeros)

    def finalize(out_arrs):
        return [
            {name: np.asarray(out_arrs[i]).reshape(NCORES, *out_avals[i].shape)[c]
             for i, name in enumerate(out_names)}
            for c in range(NCORES)
        ]

    def run(in_maps):
        return finalize(dispatch(in_maps))

    run.dispatch = dispatch
    run.finalize = finalize
    run.dev_cache = dev_cache
    return run


# ------------------------------------------------------------------- kernel

def _reduce_out(results, nmask):
    total = sum(float(np.asarray(r["msepart"]).sum()) for r in results)
    return np.float32(total / (nmask * IN_DIM))


def kernel(attr, src, dst, mask_idx, W0, al0, ar0, b0, W1, al1, ar1, b1, Wd, bd, mask_token):
    attr = np.asarray(attr)
    src_r = np.asarray(src)
    dst_r = np.asarray(dst)
    mask_r = np.asarray(mask_idx)

    def fps():
        gfp = _fingerprint(src_r, dst_r, mask_r)
        afp = _fingerprint(attr, np.asarray(W0), np.asarray(W1), np.asarray(Wd),
                           np.asarray(al0), np.asarray(ar0), np.asarray(al1),
                           np.asarray(ar1), np.asarray(b0), np.asarray(b1),
                           np.asarray(bd), np.asarray(mask_token))
        return gfp, afp

    # warm path: dispatch asynchronously with cached device inputs, verify
    # input fingerprints while the device runs, fall back on mismatch
    run = _CACHE.get("run")
    if run is not None and "in_maps" in _CACHE and \
            run.dev_cache.get("key") == id(_CACHE["in_maps"]):
        out_arrs = run.dispatch(_CACHE["in_maps"])
        gfp, afp = fps()
        if gfp == _CACHE.get("gfp") and afp == _CACHE.get("afp"):
            return _reduce_out(run.finalize(out_arrs), mask_r.shape[0])
    else:
        gfp, afp = fps()

    # cold / changed-input path
    attr32 = np.asarray(attr, dtype=np.float32)
    src64 = src_r.astype(np.int64)
    dst64 = dst_r.astype(np.int64)
    mask64 = mask_r.astype(np.int64)

    if _CACHE.get("gfp") != gfp or "graph" not in _CACHE:
        _CACHE["gfp"] = gfp
        _CACHE["graph"] = _prep_graph(src64, dst64, mask64)
        _CACHE.pop("afp", None)
    g = _CACHE["graph"]

    key = (g["T"], g["TM"])
    if _CACHE.get("progkey") != key:
        nc = _build_program(list(g["T"]), g["TT"], g["TM"])
        _CACHE["run"] = _get_runner(nc)
        _CACHE["progkey"] = key

    if _CACHE.get("afp") != afp or "in_maps" not in _CACHE:
        _CACHE["afp"] = afp
        _CACHE["in_maps"] = _prep_inputs(
            attr32, mask64, W0, al0, ar0, b0, W1, al1, ar1, b1, Wd, bd,
            mask_token, g)

    return _reduce_out(_CACHE["run"](_CACHE["in_maps"]), mask_r.shape[0])
